# revision 7
# baseline (speedup 1.0000x reference)
"""GAT network on 8 TRN2 NeuronCores — full on-device message passing.

Sharding: nodes split into 8 contiguous ranges (6250/core); edges sharded by
dst and sorted by dst, so the segment softmax and aggregation stay
core-local. Per layer and per 127-node window, source rows [h | e_s] are
gathered with Ant dma_gather (bf16 rows), e_d rows are gathered from the
core-local dst table, scores e = lrelu(e_s+e_d) are exponentiated raw (the
score range ~0.4 makes the reference's segment-max shift a mathematical
no-op), and psum[seg, :] += S_block^T @ [w*h_src | w] via one-hot segment
matmuls. The window epilogue divides by the weight sums, applies bias+ELU,
and fuses the next layer's dense projection (W plus folded score
projections A = W @ a_flat), emitting the next gather-table slice;
slices are AllGathered between layers. Graph mean-pool reuses the one-hot
matmul per window into a 512-graph accumulator, AllReduce combines node
shards, and every core runs the fc head + log_softmax on all 512 graphs
(host reads core 0).

dma_gather indices are int16, so src gathers run as two passes per window
against the low [0,32768) and high [32768,N) table halves. Per-(window,
half) block counts are padded to uniform K_LO/K_HI so all cores run one
SPMD program; pad edges use valid dummy indices and a trash segment (127).
"""

import sys

for p in ("/opt/trn_rl_repo", "/opt/trn_rl_repo/concourse"):
    if p not in sys.path:
        sys.path.insert(0, p)

import numpy as np
import ml_dtypes

import concourse.bass as bass
import concourse.mybir as mybir
import concourse.tile as tile
from concourse import bacc
from concourse.bass_utils import run_bass_kernel_spmd

f32 = mybir.dt.float32
bf16 = mybir.dt.bfloat16
i16 = mybir.dt.int16

N_CORES = 8
ABLATE = ""
NEG_SLOPE = 0.2
HALF = 32768  # int16 index limit
WIN = 127  # nodes per window (seg 127 = trash)


def _wrap16(idx):
    """dma_gather index layout: element i -> [i % 16, i // 16]."""
    n = len(idx)
    assert n % 16 == 0
    return np.ascontiguousarray(idx.reshape(n // 16, 16).T)


def prep(x, edge_index, batch, weights, n_graphs):
    x = np.asarray(x, np.float32)
    n = x.shape[0]
    assert n % N_CORES == 0
    npc = n // N_CORES
    nw = (npc + WIN - 1) // WIN

    ei = np.asarray(edge_index)
    loop = np.arange(n, dtype=ei.dtype)
    src = np.concatenate([ei[0], loop]).astype(np.int64)
    dst = np.concatenate([ei[1], loop]).astype(np.int64)
    order = np.argsort(dst, kind="stable")
    src, dst = src[order], dst[order]

    bounds = np.arange(0, nw * WIN * N_CORES + 1, WIN)
    # window w of core c covers nodes [c*npc + w*WIN, +span); global window
    # boundaries are c*npc + w*WIN which is NOT uniform in WIN steps across
    # cores (npc % WIN != 0) -> compute per (c, w) via searchsorted.
    k_lo = k_hi = 0
    lists = []
    for c in range(N_CORES):
        for w in range(nw):
            nb = c * npc + w * WIN
            ne = min(nb + WIN, (c + 1) * npc)
            s0, s1 = np.searchsorted(dst, (nb, ne))
            sl, dl = src[s0:s1], dst[s0:s1]
            lo = sl < HALF
            lists.append((sl[lo], dl[lo], sl[~lo], dl[~lo]))
            k_lo = max(k_lo, (int(lo.sum()) + 127) // 128)
            k_hi = max(k_hi, (int((~lo).sum()) + 127) // 128)

    nblk = k_lo + k_hi
    maxcnt = np.zeros((nw, 2), np.int64)
    for c in range(N_CORES):
        for w in range(nw):
            lo_s, _, hi_s, _ = lists[c * nw + w]
            maxcnt[w, 0] = max(maxcnt[w, 0], len(lo_s))
            maxcnt[w, 1] = max(maxcnt[w, 1], len(hi_s))
    sidx = np.zeros((N_CORES, 16, nw * nblk * 8), np.int16)
    didx = np.zeros((N_CORES, 16, nw * nblk * 8), np.int16)
    seg = np.full((N_CORES, 128, nw * nblk), 127, np.float32)
    for c in range(N_CORES):
        for w in range(nw):
            lo_s, lo_d, hi_s, hi_d = lists[c * nw + w]
            base_node = c * npc + w * WIN
            for ss, dd, kk, boff in (
                (lo_s, lo_d, k_lo, 0),
                (hi_s - HALF, hi_d, k_hi, k_lo),
            ):
                m = len(ss)
                cap = kk * 128
                sp = np.zeros(cap, np.int64)
                dp = np.zeros(cap, np.int64)
                sg = np.full(cap, 127, np.int64)
                sp[:m] = ss
                dp[:m] = dd - c * npc
                sg[:m] = dd - base_node
                b0 = w * nblk + boff
                sidx[c, :, b0 * 8 : (b0 + kk) * 8] = _wrap16(sp.astype(np.int16))
                didx[c, :, b0 * 8 : (b0 + kk) * 8] = _wrap16(dp.astype(np.int16))
                seg[c, :, b0 : b0 + kk] = sg.reshape(kk, 128).T.astype(np.float32)

    # L1 tables on host (2->64 projection is trivial)
    W1, a1s, a1d = weights["W1"], weights["a1s"], weights["a1d"]
    H1, C1 = a1s.shape
    F1 = H1 * C1
    h1 = (x @ W1).astype(np.float32)
    es1 = (h1.reshape(n, H1, C1) * a1s).sum(-1)
    ed1 = (h1.reshape(n, H1, C1) * a1d).sum(-1)
    T1 = np.zeros((n, 128), np.float32)
    T1[:, :F1] = h1
    T1[:, F1 : F1 + 8] = es1
    D1f = np.zeros((n, 128), np.float32)
    D1f[:, :8] = ed1

    def aug(W, a_s, a_d):
        Hh, Cc = a_s.shape
        Fo = W.shape[1]
        asf = np.zeros((Fo, 8), np.float32)
        adf = np.zeros((Fo, 8), np.float32)
        for h in range(Hh):
            asf[h * Cc : (h + 1) * Cc, h] = a_s[h]
            adf[h * Cc : (h + 1) * Cc, h] = a_d[h]
        return np.concatenate([W, W @ asf, W @ adf], axis=1).astype(np.float32)

    W2aug = aug(weights["W2"], weights["a2s"], weights["a2d"])
    W3aug = aug(weights["W3"], weights["a3s"], weights["a3d"])
    F2 = weights["W2"].shape[1]
    F3 = weights["W3"].shape[1]

    batch = np.asarray(batch).astype(np.int64)
    cnt = np.bincount(batch, minlength=n_graphs).astype(np.float32)
    gbufs = (n_graphs + 127) // 128
    gseg4 = np.full((N_CORES, 128, nw, gbufs), -1.0, np.float32)
    for c in range(N_CORES):
        for w in range(nw):
            nb = c * npc + w * WIN
            ne = min(nb + WIN, (c + 1) * npc)
            col = np.full(128, -1.0, np.float32)
            col[: ne - nb] = batch[nb:ne]
            for b in range(gbufs):
                gseg4[c, :, w, b] = col - 128.0 * b
                gseg4[c, col < 0, w, b] = -1.0

    kmax = max(k_lo, k_hi, gbufs)
    iota = np.arange(128, dtype=np.float32)
    iota_rep = np.ascontiguousarray(
        np.tile(np.broadcast_to(iota, (128, 128)), (1, kmax))
    )

    dims = dict(
        n=n, npc=npc, nw=nw, k_lo=k_lo, k_hi=k_hi, nblk=nblk, kmax=kmax,
        F=[F1, F2, F3], n_graphs=n_graphs, gbufs=gbufs, maxcnt=maxcnt,
        spans=[min(WIN, npc - w * WIN) for w in range(nw)],
    )

    shared = {
        "T1lo": T1[:HALF].astype(ml_dtypes.bfloat16),
        "T1hi": T1[HALF:].astype(ml_dtypes.bfloat16),
        "iota_rep": iota_rep.astype(ml_dtypes.bfloat16),
        "iota_f32": np.ascontiguousarray(iota_rep[:, : gbufs * 128]),
        "W2aug": W2aug.astype(ml_dtypes.bfloat16),
        "W3aug": W3aug.astype(ml_dtypes.bfloat16),
        "b1r": np.broadcast_to(weights["b1"], (128, F1)).copy(),
        "b2r": np.broadcast_to(weights["b2"], (128, F2)).copy(),
        "b3r": np.broadcast_to(weights["b3"], (128, F3)).copy(),
        "ident": np.eye(128, dtype=np.float32),
        "fc1W": weights["fc1W"].astype(np.float32),
        "fc2W": weights["fc2W"].astype(np.float32),
        "b1h": np.broadcast_to(weights["fc1b"], (128, 32)).copy(),
        "b2h": np.broadcast_to(weights["fc2b"], (128, 10)).copy(),
    }
    per_core = []
    for c in range(N_CORES):
        im = {
            **shared,
            "D1": D1f[c * npc : (c + 1) * npc].astype(ml_dtypes.bfloat16),
            "icnt": (1.0 / np.maximum(
                cnt[c * (n_graphs // N_CORES) : (c + 1) * (n_graphs // N_CORES)], 1.0
            )).reshape(-1, 1).astype(np.float32),
            "sidx": np.ascontiguousarray(np.tile(sidx[c], (8, 1))),
            "didx": np.ascontiguousarray(np.tile(didx[c], (8, 1))),
            "seg": np.ascontiguousarray(seg[c]).astype(ml_dtypes.bfloat16),
            "gseg4": np.ascontiguousarray(gseg4[c].reshape(128, nw * gbufs)),
        }
        per_core.append(im)
    return dims, per_core


def build(dims, nw_limit=None):
    d = dims
    nw, k_lo, k_hi, nblk, kmax = d["nw"], d["k_lo"], d["k_hi"], d["nblk"], d["kmax"]
    npc = d["npc"]
    F = d["F"]
    n = d["n"]
    ng = d["n_graphs"]
    gbufs = d["gbufs"]

    nc = bacc.Bacc(None, target_bir_lowering=False)

    T1lo = nc.dram_tensor("T1lo", [HALF, 128], bf16, kind="ExternalInput")
    T1hi = nc.dram_tensor("T1hi", [n - HALF, 128], bf16, kind="ExternalInput")
    D1 = nc.dram_tensor("D1", [npc, 128], bf16, kind="ExternalInput")
    sidx_d = nc.dram_tensor("sidx", [128, nw * nblk * 8], i16, kind="ExternalInput")
    didx_d = nc.dram_tensor("didx", [128, nw * nblk * 8], i16, kind="ExternalInput")
    seg_d = nc.dram_tensor("seg", [128, nw * nblk], bf16, kind="ExternalInput")
    gseg4_d = nc.dram_tensor("gseg4", [128, nw * gbufs], f32, kind="ExternalInput")
    iota_d = nc.dram_tensor("iota_rep", [128, kmax * 128], bf16, kind="ExternalInput")
    iotaf_d = nc.dram_tensor("iota_f32", [128, gbufs * 128], f32, kind="ExternalInput")
    W2aug_d = nc.dram_tensor("W2aug", [F[0], F[1] + 16], bf16, kind="ExternalInput")
    W3aug_d = nc.dram_tensor("W3aug", [F[1], F[2] + 16], bf16, kind="ExternalInput")
    b_d = [
        nc.dram_tensor("b1r", [128, F[0]], f32, kind="ExternalInput"),
        nc.dram_tensor("b2r", [128, F[1]], f32, kind="ExternalInput"),
        nc.dram_tensor("b3r", [128, F[2]], f32, kind="ExternalInput"),
    ]
    ident_d = nc.dram_tensor("ident", [128, 128], f32, kind="ExternalInput")
    fc1W_d = nc.dram_tensor("fc1W", [F[2], 32], f32, kind="ExternalInput")
    fc2W_d = nc.dram_tensor("fc2W", [32, 10], f32, kind="ExternalInput")
    b1h_d = nc.dram_tensor("b1h", [128, 32], f32, kind="ExternalInput")
    b2h_d = nc.dram_tensor("b2h", [128, 10], f32, kind="ExternalInput")
    icnt_d = nc.dram_tensor("icnt", [ng // N_CORES, 1], f32, kind="ExternalInput")
    out_d = nc.dram_tensor("out", [ng // N_CORES, 10], f32, kind="ExternalOutput")

    ag2_src = nc.dram_tensor("ag2_src", [npc, 256], bf16)
    ag3_src = nc.dram_tensor("ag3_src", [npc, 256], bf16)
    T2 = nc.dram_tensor("T2", [n, 256], bf16, addr_space="Shared")
    T3 = nc.dram_tensor("T3", [n, 256], bf16, addr_space="Shared")
    T2hi = nc.dram_tensor("T2hi", [n - HALF, 256], bf16)
    T3hi = nc.dram_tensor("T3hi", [n - HALF, 256], bf16)
    D2 = nc.dram_tensor("D2", [npc, 128], bf16)
    D3 = nc.dram_tensor("D3", [npc, 128], bf16)
    gpc = ng // N_CORES
    gsum_l = nc.dram_tensor("gsum_l", [gbufs * 128, 128], f32)
    gsum_a = nc.dram_tensor("gsum_a", [ng // N_CORES, 128], f32)

    rg = [list(range(N_CORES))]

    with tile.TileContext(nc) as tc:
        with (
            tc.tile_pool(name="const", bufs=1) as cp,
            tc.tile_pool(name="gp", bufs=4) as gp,
            tc.tile_pool(name="sp", bufs=4) as sp,
            tc.tile_pool(name="ep", bufs=4) as ep,
            tc.tile_pool(name="psum", bufs=4, space="PSUM") as pp,
            tc.tile_pool(name="psum2", bufs=2, space="PSUM") as pp2,
        ):
            sidx_s = cp.tile([128, nw * nblk * 8], i16)
            didx_s = cp.tile([128, nw * nblk * 8], i16)
            seg_s = cp.tile([128, nw * nblk], bf16)
            gseg4_s = cp.tile([128, nw * gbufs], f32)
            iota_s = cp.tile([128, kmax * 128], bf16)
            iotaf_s = cp.tile([128, gbufs * 128], f32)
            W2aug_s = cp.tile([F[0], F[1] + 16], bf16)
            W3aug_s = cp.tile([F[1], F[2] + 16], bf16)
            ident_s = cp.tile([128, 128], f32)
            b0_s = cp.tile([128, F[0]], f32, tag="bias0")
            b1_s = cp.tile([128, F[1]], f32, tag="bias1")
            b2_s = cp.tile([128, F[2]], f32, tag="bias2")
            b_s = [b0_s, b1_s, b2_s]
            gacc = cp.tile([128, gbufs, 128], f32)
            for t, src_t in (
                (sidx_s, sidx_d), (didx_s, didx_d), (seg_s, seg_d),
                (gseg4_s, gseg4_d), (iota_s, iota_d), (iotaf_s, iotaf_d),
                (W2aug_s, W2aug_d), (W3aug_s, W3aug_d), (ident_s, ident_d),
            ):
                nc.sync.dma_start(out=t[:], in_=src_t[:])
            for i in range(3):
                nc.sync.dma_start(out=b_s[i][:], in_=b_d[i][:])
            nc.vector.memset(gacc[:], 0.0)

            Waug_next = {0: W2aug_s, 1: W3aug_s}
            D_next = {0: D2, 1: D3}
            ag_next = {0: ag2_src, 1: ag3_src}

            def layer(li, T_lo, T_hi, D_src):
                FI = F[li]
                C = FI // 8
                row = 128 if li == 0 else 256
                for w in range(nw if nw_limit is None else min(nw, nw_limit)):
                    span = d["spans"][w]
                    psw = pp.tile([128, FI + 8], f32, tag="win")
                    first = True
                    halves = [(k_lo, 0, 0), (k_hi, k_lo, HALF)]
                    halves = [h for h in halves if h[0] > 0]
                    for hi_i, (kk, boff, base) in enumerate(halves):
                        b0 = w * nblk + boff
                        tbl = T_lo[0:HALF, :] if base == 0 else T_hi[:, :]
                        g = gp.tile([128, kmax, row], bf16, tag=f"g{li}")
                        ed = gp.tile([128, kmax, 128], bf16, tag="ed")
                        mreal = int(d["maxcnt"][w][1 if base else 0])
                        if w < 2:
                            mreal = kk * 128
                        for j0 in range(0, kk, 4):
                            jk = min(4, kk - j0)
                            nfetch = min(jk * 128, mreal - j0 * 128)
                            if nfetch <= 0:
                                continue
                            jv = (nfetch + 127) // 128
                            nc.gpsimd.dma_gather(
                                g[:, j0 : j0 + jv, :],
                                tbl,
                                sidx_s[:, (b0 + j0) * 8 : (b0 + j0 + jv) * 8],
                                nfetch, nfetch, row,
                            )
                            nc.gpsimd.dma_gather(
                                ed[:, j0 : j0 + jv, :],
                                D_src[:, :],
                                didx_s[:, (b0 + j0) * 8 : (b0 + j0 + jv) * 8],
                                nfetch, nfetch, 128,
                            )
                        S = sp.tile([128, kmax * 128], bf16, tag="S")
                        if "dve" not in ABLATE:
                         nc.vector.tensor_tensor(
                            out=S[:, 0 : kk * 128].rearrange(
                                "p (k j) -> p k j", j=128
                            ),
                            in0=iota_s[:, 0 : kk * 128].rearrange(
                                "p (k j) -> p k j", j=128
                            ),
                            in1=seg_s[:, b0 : b0 + kk, None].to_broadcast(
                                [128, kk, 128]
                            ),
                            op=mybir.AluOpType.is_equal,
                        )
                        e = ep.tile([128, kmax, 8], f32, tag="e")
                        nc.vector.tensor_tensor(
                            out=e[:, 0:kk, :],
                            in0=g[:, 0:kk, FI : FI + 8],
                            in1=ed[:, 0:kk, 0:8],
                            op=mybir.AluOpType.add,
                        )
                        et = ep.tile([128, kmax, 8], f32, tag="et")
                        nc.vector.tensor_scalar(
                            et[:, 0:kk, :], e[:, 0:kk, :], 0.0, NEG_SLOPE,
                            mybir.AluOpType.min, mybir.AluOpType.mult,
                        )
                        nc.vector.tensor_scalar_max(
                            e[:, 0:kk, :], e[:, 0:kk, :], 0.0
                        )
                        nc.vector.tensor_add(
                            e[:, 0:kk, :], e[:, 0:kk, :], et[:, 0:kk, :]
                        )
                        mw = sp.tile([128, kmax, 8, C + 1], bf16, tag="mw")
                        nc.scalar.activation(
                            mw[:, 0:kk, :, C : C + 1],
                            e[:, 0:kk, :, None],
                            mybir.ActivationFunctionType.Exp,
                        )
                        nc.vector.tensor_tensor(
                            out=mw[:, 0:kk, :, 0:C],
                            in0=g[:, 0:kk, 0:FI].rearrange(
                                "p k (h c) -> p k h c", c=C
                            ),
                            in1=mw[:, 0:kk, :, C : C + 1].to_broadcast(
                                [128, kk, 8, C]
                            ),
                            op=mybir.AluOpType.mult,
                        )
                        for b in range(kk if "mm" not in ABLATE else 1):
                            nc.tensor.matmul(
                                out=psw[:, :],
                                lhsT=S[:, b * 128 : (b + 1) * 128],
                                rhs=mw[:, b].rearrange("p h c -> p (h c)"),
                                start=first,
                                stop=(hi_i == len(halves) - 1) and b == kk - 1,
                            )
                            first = False
                    # window epilogue; psum cols interleaved [h0(C)|w0|h1(C)|w1...]
                    psw3 = psw[:].rearrange("p (h c) -> p h c", c=C + 1)
                    sc = ep.tile([128, 8], f32, tag="sc")
                    nc.vector.tensor_scalar_max(
                        sc[:], psw3[:, :, C], 1e-30
                    )
                    rt = ep.tile([128, 8], f32, tag="rt")
                    nc.vector.reciprocal(rt[:], sc[:])
                    hn = ep.tile([128, FI], f32, tag="hn")
                    nc.vector.tensor_tensor(
                        out=hn[:].rearrange("p (h c) -> p h c", c=C),
                        in0=psw3[:, :, 0:C],
                        in1=rt[:, :, None].to_broadcast([128, 8, C]),
                        op=mybir.AluOpType.mult,
                    )
                    nc.vector.tensor_add(hn[:], hn[:], b_s[li][:])
                    if li < 2:
                        t1 = ep.tile([128, FI], f32, tag="t1")
                        nc.vector.tensor_scalar_min(t1[:], hn[:], 0.0)
                        nc.scalar.activation(
                            t1[:], t1[:], mybir.ActivationFunctionType.Exp
                        )
                        nc.vector.tensor_scalar(
                            hn[:], hn[:], 0.0, -1.0,
                            mybir.AluOpType.max, mybir.AluOpType.add,
                        )
                        nc.vector.tensor_add(hn[:], hn[:], t1[:])
                        FO = F[li + 1]
                        pst = pp2.tile([128, 128], f32, tag="ptrans")
                        nc.tensor.transpose(
                            out=pst[0:FI, :], in_=hn[:], identity=ident_s[:]
                        )
                        hT = ep.tile([128, 128], bf16, tag="hT")
                        nc.scalar.activation(
                            hT[0:FI, :], pst[0:FI, :],
                            mybir.ActivationFunctionType.Copy,
                        )
                        ps2 = pp2.tile([128, FO + 16], f32, tag="pdense")
                        nc.tensor.matmul(
                            out=ps2[:, :],
                            lhsT=hT[0:FI, :],
                            rhs=Waug_next[li][:],
                            start=True, stop=True,
                        )
                        ttile = ep.tile([128, 256], bf16, tag="ttile")
                        nc.vector.memset(ttile[:, FO + 8 : 256], 0.0)
                        nc.scalar.activation(
                            ttile[:, 0 : FO + 8], ps2[:, 0 : FO + 8],
                            mybir.ActivationFunctionType.Copy,
                        )
                        dtile = ep.tile([128, 128], bf16, tag="dtile")
                        nc.vector.memset(dtile[:, 8:128], 0.0)
                        nc.scalar.activation(
                            dtile[:, 0:8], ps2[:, FO + 8 : FO + 16],
                            mybir.ActivationFunctionType.Copy,
                        )
                        nc.sync.dma_start(
                            out=ag_next[li][w * WIN : w * WIN + span, :],
                            in_=ttile[0:span, :],
                        )
                        nc.sync.dma_start(
                            out=D_next[li][w * WIN : w * WIN + span, :],
                            in_=dtile[0:span, :],
                        )
                    else:
                        Sg = sp.tile([128, gbufs * 128], f32, tag="Sg")
                        nc.vector.tensor_tensor(
                            out=Sg[:].rearrange("p (k j) -> p k j", j=128),
                            in0=iotaf_s[:].rearrange("p (k j) -> p k j", j=128),
                            in1=gseg4_s[
                                :, w * gbufs : (w + 1) * gbufs, None
                            ].to_broadcast([128, gbufs, 128]),
                            op=mybir.AluOpType.is_equal,
                        )
                        for b in range(gbufs):
                            ps3 = pp2.tile([128, 128], f32, tag="pdense")
                            nc.tensor.matmul(
                                out=ps3[:, 0:FI],
                                lhsT=Sg[:, b * 128 : (b + 1) * 128],
                                rhs=hn[:],
                                start=True, stop=True,
                            )
                            nc.vector.tensor_add(
                                gacc[:, b, :], gacc[:, b, :], ps3[:, 0:FI]
                            )

            layer(0, T1lo, T1hi, D1)
            nc.gpsimd.collective_compute(
                "AllGather", mybir.AluOpType.bypass,
                replica_groups=rg, ins=[ag2_src[:]], outs=[T2[:]],
            )
            nc.sync.dma_start(out=T2hi[:], in_=T2[HALF:n, :])
            layer(1, T2[0:HALF, :].tensor if False else T2, T2hi, D2)
            nc.gpsimd.collective_compute(
                "AllGather", mybir.AluOpType.bypass,
                replica_groups=rg, ins=[ag3_src[:]], outs=[T3[:]],
            )
            nc.sync.dma_start(out=T3hi[:], in_=T3[HALF:n, :])
            layer(2, T3, T3hi, D3)

            for b in range(gbufs):
                nc.sync.dma_start(
                    out=gsum_l[b * 128 : (b + 1) * 128, :], in_=gacc[:, b, :]
                )
            nc.gpsimd.collective_compute(
                "ReduceScatter", mybir.AluOpType.add,
                replica_groups=rg, ins=[gsum_l[0:ng, :]], outs=[gsum_a[:]],
            )

            # head: all graphs on every core, chunks of 128
            b1h_s = ep.tile([gpc, 32], f32, tag="b1h")
            nc.sync.dma_start(out=b1h_s[:], in_=b1h_d[0:gpc, :])
            b2h_s = ep.tile([gpc, 10], f32, tag="b2h")
            nc.sync.dma_start(out=b2h_s[:], in_=b2h_d[0:gpc, :])
            fc1W_s = ep.tile([F[2], 32], f32, tag="fc1W")
            nc.sync.dma_start(out=fc1W_s[:], in_=fc1W_d[:])
            fc2W_s = ep.tile([32, 10], f32, tag="fc2W")
            nc.sync.dma_start(out=fc2W_s[:], in_=fc2W_d[:])
            for b in range(1):
                pooled = ep.tile([gpc, F[2]], f32, tag="pooled")
                nc.sync.dma_start(out=pooled[:], in_=gsum_a[0:gpc, :])
                icnt_s = ep.tile([gpc, 1], f32, tag="icnt")
                nc.sync.dma_start(out=icnt_s[:], in_=icnt_d[0:gpc, :])
                nc.vector.tensor_scalar_mul(pooled[:], pooled[:], icnt_s[:, 0:1])
                psT = pp2.tile([128, gpc], f32, tag="ptrans")
                nc.tensor.transpose(
                    out=psT[0 : F[2], :], in_=pooled[:],
                    identity=ident_s[0:gpc, 0:gpc],
                )
                pT = ep.tile([F[2], gpc], f32, tag="pT")
                nc.vector.tensor_copy(out=pT[:], in_=psT[0 : F[2], :])
                z1p = pp2.tile([gpc, 32], f32, tag="pdense")
                nc.tensor.matmul(
                    out=z1p[:], lhsT=pT[:], rhs=fc1W_s[:], start=True, stop=True
                )
                z1 = ep.tile([gpc, 32], f32, tag="z1")
                nc.vector.tensor_add(z1[:], z1p[:], b1h_s[:])
                nc.vector.tensor_scalar_max(z1[:], z1[:], 0.0)
                psT2 = pp2.tile([32, gpc], f32, tag="ptrans")
                nc.tensor.transpose(out=psT2[:], in_=z1[:], identity=ident_s[0:gpc, 0:gpc])
                z1T = ep.tile([32, gpc], f32, tag="z1T")
                nc.vector.tensor_copy(out=z1T[:], in_=psT2[:])
                zp = pp2.tile([gpc, 10], f32, tag="pdense")
                nc.tensor.matmul(
                    out=zp[:], lhsT=z1T[:], rhs=fc2W_s[:], start=True, stop=True
                )
                z = ep.tile([gpc, 10], f32, tag="z")
                nc.vector.tensor_add(z[:], zp[:], b2h_s[:])
                mneg = ep.tile([gpc, 1], f32, tag="mneg")
                nc.vector.tensor_reduce(
                    mneg[:], z[:], mybir.AxisListType.X, mybir.AluOpType.max
                )
                nc.vector.tensor_scalar_mul(mneg[:], mneg[:], -1.0)
                eb = ep.tile([gpc, 10], f32, tag="eb")
                sb = ep.tile([gpc, 1], f32, tag="sb")
                nc.scalar.activation(
                    eb[:], z[:], mybir.ActivationFunctionType.Exp,
                    bias=mneg[:, 0:1], accum_out=sb[:, 0:1],
                )
                nls = ep.tile([gpc, 1], f32, tag="nls")
                nc.scalar.activation(
                    nls[:], sb[:], mybir.ActivationFunctionType.Ln
                )
                nc.vector.tensor_scalar_mul(nls[:], nls[:], -1.0)
                ob = ep.tile([gpc, 10], f32, tag="ob")
                nc.vector.tensor_scalar(
                    ob[:], z[:], mneg[:, 0:1], nls[:, 0:1],
                    mybir.AluOpType.add, mybir.AluOpType.add,
                )
                nc.sync.dma_start(out=out_d[:], in_=ob[:])

    nc.finalize()
    return nc


def run(x, edge_index, batch, weights, n_graphs=512):
    dims, per_core = prep(x, edge_index, batch, weights, n_graphs)
    nc = build(dims)
    res = run_bass_kernel_spmd(nc, per_core, list(range(N_CORES)))
    out = np.concatenate(
        [np.asarray(res.results[c]["out"], np.float32) for c in range(N_CORES)]
    )
    return out[:n_graphs], dims, nc


def kernel(x, edge_index, batch, W1, a1s, a1d, b1, W2, a2s, a2d, b2, W3, a3s,
           a3d, b3, fc1W, fc1b, fc2W, fc2b):
    weights = dict(
        W1=np.asarray(W1, np.float32), a1s=np.asarray(a1s, np.float32),
        a1d=np.asarray(a1d, np.float32), b1=np.asarray(b1, np.float32),
        W2=np.asarray(W2, np.float32), a2s=np.asarray(a2s, np.float32),
        a2d=np.asarray(a2d, np.float32), b2=np.asarray(b2, np.float32),
        W3=np.asarray(W3, np.float32), a3s=np.asarray(a3s, np.float32),
        a3d=np.asarray(a3d, np.float32), b3=np.asarray(b3, np.float32),
        fc1W=np.asarray(fc1W, np.float32), fc1b=np.asarray(fc1b, np.float32),
        fc2W=np.asarray(fc2W, np.float32), fc2b=np.asarray(fc2b, np.float32),
    )
    out, _, _ = run(x, edge_index, batch, weights, 512)
    return out
